# revision 4
# baseline (speedup 1.0000x reference)
"""Trainium2 Bass kernel for nn_Conv2d (B=32, 256->256, 56x56, 3x3, pad=1) + bias.

Strategy
--------
Data-parallel over batch: 4 images per NeuronCore x 8 cores; weights/bias
replicated; no collectives.

Per core the conv is computed as shifted matmuls (as in the fp32r baseline:
zero-padded 58-wide rows, output position (h,w) <-> flat h*58+w, tap (kh,kw)
is a matmul against the input shifted by kh*58+kw), but the matmuls run as
fp8 e4m3 DoubleRow (2 fp8 weights per PE cell, 2 MACs/cycle -> 2x the fp32r
FLOP rate).

Precision: e4m3 alone is too coarse (measured 3.1e-2 rel err vs the 2e-2
gate).  The host splits each operand into hi + lo e4m3 parts at one shared
scale (X~=16x as xh+xl, W~=256w as wh+wl) and the kernel adds first-order
compensation terms (wh*xl + wl*xh) on 12 of the 18 (cin-chunk, tap) chunks.
DoubleRow contracts two 128-chunks per pass, so per output tile:
  9 main passes   = wh*xh over all 18 chunks (ci pairs share a pass)
  12 comp passes  = (wl*xh + wh*xl) for the 12 covered chunks
= 21 DoubleRow matmuls vs 18 full-rate fp32r equivalents.  Measured rel err
(exact, deterministic inputs): 1.79e-2.

PSUM accumulates all 21 passes at scale 4096; eviction fuses the unscale and
bias add on ScalarE: out = Identity(psum/4096 + bias).  Waves of 4+3 PSUM
tiles share each weight pass across the wave so the per-pass LDWEIGHTS
(~213ns, 256 cols, no FWL in DoubleRow) hides under 3-4 matmuls (~109ns
each) via the PE background weight buffer.
"""

import numpy as np
import ml_dtypes

import concourse.bacc as bacc
import concourse.tile as tile
import concourse.mybir as mybir
from concourse.bass_utils import run_bass_kernel_spmd

F32 = mybir.dt.float32
F8 = mybir.dt.float8e4
DR = mybir.MatmulPerfMode.DoubleRow
IDENT = mybir.ActivationFunctionType.Identity

B, CIN, COUT, H, W, K = 32, 256, 256, 56, 56, 3
NCORES = 8
BPC = B // NCORES          # images per core
WP = W + 2                 # padded row width (58)
HP = H + 3                 # padded rows (59): 1 top, 2 bottom (tail tap reads)
XF = HP * WP               # valid padded flat length per plane (3422)
XFP = 3424                 # plane pitch in SBUF/DRAM (16B aligned for DoubleRow)
OF = H * WP                # padded output flat length (3248)
NT = 7                     # output tiles per (img, cout-chunk)
NFREE = OF // NT           # 464 positions per matmul
XLOAD = 3366               # matmuls never read past 3365

SX = np.float32(16.0)      # X pre-scale (keeps e4m3 out of subnormals)
SW = np.float32(256.0)     # W pre-scale
INV_SCALE = float(1.0 / (SX * SW))

# chunks (ci, tap) with first-order fp8 compensation; 12 of 18 -> 1.79e-2
COMP = [(0, t) for t in range(9)] + [(1, t) for t in range(3)]
# 9 main DoubleRow passes (tap t, ci 0+1 in the two k-slots) + 12 comp passes
PAIRS = [("m", t) for t in range(9)] + [("c", ci, t) for (ci, t) in COMP]
NPAIR = len(PAIRS)         # 21

_CACHE = {}


def _tap_off(t):
    return (t // 3) * WP + (t % 3)


def _build():
    if "nc" in _CACHE:
        return _CACHE["nc"]
    nc = bacc.Bacc("TRN2", target_bir_lowering=False, debug=False,
                   num_swdge_queues=4)
    x_d = nc.dram_tensor("x", [BPC, 128, 4, XFP], F8, kind="ExternalInput").ap()
    w_d = nc.dram_tensor("w", [128, 2, NPAIR, 2, 128], F8,
                         kind="ExternalInput").ap()
    b_d = nc.dram_tensor("b", [COUT], F32, kind="ExternalInput").ap()
    o_d = nc.dram_tensor("o", [BPC, COUT, OF], F32, kind="ExternalOutput").ap()

    with tile.TileContext(nc) as tc:
        with (
            tc.tile_pool(name="wp", bufs=1) as wp,
            tc.tile_pool(name="xp", bufs=2) as xp,
            tc.tile_pool(name="op", bufs=2) as op,
            tc.tile_pool(name="pp", bufs=8, space="PSUM") as pp,
        ):
            bias_t = wp.tile([128, 2], F32)
            w_t = wp.tile([128, 2, NPAIR, 2, 128], F8)

            # weights + bias up front on scalar (HWDGE; idle until evictions
            # start); main-pass weights of cc=0 first so the PE can start as
            # soon as the first x slice lands
            nc.scalar.dma_start(out=w_t[:, 0:1, 0:9], in_=w_d[:, 0:1, 0:9])
            nc.scalar.dma_start(out=bias_t[:, 0:1], in_=b_d[0:128])
            nc.scalar.dma_start(out=w_t[:, 0:1, 9:NPAIR], in_=w_d[:, 0:1, 9:NPAIR])
            nc.scalar.dma_start(out=bias_t[:, 1:2], in_=b_d[128:256])
            nc.scalar.dma_start(out=w_t[:, 1:2], in_=w_d[:, 1:2])

            def load_img(img):
                # hi planes first (main passes run first), in two col slices
                x_t = xp.tile([128, 4, XFP], F8, tag="x")
                e = nc.sync
                e.dma_start(out=x_t[:, 0:2, 0:2368], in_=x_d[img, :, 0:2, 0:2368])
                e.dma_start(out=x_t[:, 2:4, 0:2368], in_=x_d[img, :, 2:4, 0:2368])
                e.dma_start(out=x_t[:, 0:2, 2368:XLOAD],
                            in_=x_d[img, :, 0:2, 2368:XLOAD])
                e.dma_start(out=x_t[:, 2:4, 2368:XLOAD],
                            in_=x_d[img, :, 2:4, 2368:XLOAD])
                return x_t

            def x_ap(x_t, p_idx, nt):
                pair = PAIRS[p_idx]
                if pair[0] == "m":
                    off = nt * NFREE + _tap_off(pair[1])
                    return x_t[:, 0:2, off:off + NFREE]          # (hi0, hi1)
                ci, t = pair[1], pair[2]
                off = nt * NFREE + _tap_off(t)
                return x_t[:, ci:ci + 3:2, off:off + NFREE]      # (hi ci, lo ci)

            WAVES = [(0, 1, 2, 3), (4, 5, 6)]
            for img in range(BPC):
                x_t = load_img(img)
                for cc in range(2):
                    o_t = op.tile([128, OF], F32, tag="o")
                    for tiles in WAVES:
                        pss = []
                        for _ in tiles:
                            ps = pp.tile([128, NFREE], F32, tag="ps")
                            pss.append(ps)
                        for p in range(NPAIR):
                            w_ap = w_t[:, cc, p]
                            for ps, nt in zip(pss, tiles):
                                nc.tensor.matmul(
                                    ps, w_ap, x_ap(x_t, p, nt),
                                    start=(p == 0), stop=(p == NPAIR - 1),
                                    perf_mode=DR,
                                )
                        for ps, nt in zip(pss, tiles):
                            # fused unscale + bias on the otherwise-idle ScalarE
                            nc.scalar.activation(
                                o_t[:, nt * NFREE:(nt + 1) * NFREE], ps, IDENT,
                                bias=bias_t[:, cc:cc + 1], scale=INV_SCALE,
                            )
                        lo = tiles[0] * NFREE
                        hi = (tiles[-1] + 1) * NFREE
                        nc.gpsimd.dma_start(
                            out=o_d[img, cc * 128:(cc + 1) * 128, lo:hi],
                            in_=o_t[:, lo:hi],
                        )
    nc.compile()
    _CACHE["nc"] = nc
    return nc


def make_in_maps(inp, kernel, bias):
    E4 = ml_dtypes.float8_e4m3
    x = np.asarray(inp, np.float32) * SX
    xpad = np.zeros((B, CIN, HP, WP), np.float32)
    xpad[:, :, 1:1 + H, 1:1 + W] = x
    xf = xpad.reshape(B, CIN, XF)
    xh8 = xf.astype(E4)
    xl8 = (xf - xh8.astype(np.float32)).astype(E4)
    xdev = np.zeros((B, 128, 4, XFP), E4)
    xdev[:, :, 0, :XF] = xh8[:, 0:128]
    xdev[:, :, 1, :XF] = xh8[:, 128:256]
    xdev[:, :, 2, :XF] = xl8[:, 0:128]
    xdev[:, :, 3, :XF] = xl8[:, 128:256]

    wsc = np.asarray(kernel, np.float32) * SW        # [cout, cin, kh, kw]
    wh8 = wsc.astype(E4)
    wl8 = (wsc - wh8.astype(np.float32)).astype(E4)

    def slab(src, ci, t, cc):
        kh, kw = divmod(t, 3)
        return src[cc * 128:(cc + 1) * 128, ci * 128:(ci + 1) * 128, kh, kw].T

    wdev = np.zeros((128, 2, NPAIR, 2, 128), E4)
    for cc in range(2):
        for p_idx, pair in enumerate(PAIRS):
            if pair[0] == "m":
                t = pair[1]
                wdev[:, cc, p_idx, 0] = slab(wh8, 0, t, cc)   # x slot: hi0
                wdev[:, cc, p_idx, 1] = slab(wh8, 1, t, cc)   # x slot: hi1
            else:
                ci, t = pair[1], pair[2]
                wdev[:, cc, p_idx, 0] = slab(wl8, ci, t, cc)  # x slot: hi ci
                wdev[:, cc, p_idx, 1] = slab(wh8, ci, t, cc)  # x slot: lo ci
    b_dev = np.ascontiguousarray(np.asarray(bias, np.float32))
    return [
        {"x": np.ascontiguousarray(xdev[c * BPC:(c + 1) * BPC]),
         "w": wdev, "b": b_dev}
        for c in range(NCORES)
    ]


def assemble(results):
    o = np.concatenate([results[c]["o"] for c in range(NCORES)], axis=0)
    return np.ascontiguousarray(
        o.reshape(B, COUT, H, WP)[:, :, :, :W].astype(np.float32)
    )


def kernel(inp, kernel, bias):
    nc = _build()
    in_maps = make_in_maps(inp, kernel, bias)
    r = run_bass_kernel_spmd(nc, in_maps, core_ids=list(range(NCORES)))
    return assemble(r.results)


# revision 5
# speedup vs baseline: 1.0003x; 1.0003x over previous
"""Trainium2 Bass kernel for nn_Conv2d (B=32, 256->256, 56x56, 3x3, pad=1) + bias.

Strategy
--------
Data-parallel over batch: 4 images per NeuronCore x 8 cores; weights/bias
replicated; no collectives.

Per core the conv is computed as shifted matmuls (as in the fp32r baseline:
zero-padded 58-wide rows, output position (h,w) <-> flat h*58+w, tap (kh,kw)
is a matmul against the input shifted by kh*58+kw), but the matmuls run as
fp8 e4m3 DoubleRow (2 fp8 weights per PE cell, 2 MACs/cycle -> 2x the fp32r
FLOP rate).

Precision: e4m3 alone is too coarse (measured 3.1e-2 rel err vs the 2e-2
gate).  The host splits each operand into hi + lo e4m3 parts at one shared
scale (X~=16x as xh+xl, W~=256w as wh+wl) and the kernel adds first-order
compensation terms (wh*xl + wl*xh) on 12 of the 18 (cin-chunk, tap) chunks.
DoubleRow contracts two 128-chunks per pass, so per output tile:
  9 main passes   = wh*xh over all 18 chunks (ci pairs share a pass)
  12 comp passes  = (wl*xh + wh*xl) for the 12 covered chunks
= 21 DoubleRow matmuls vs 18 full-rate fp32r equivalents.  Measured rel err
(exact, deterministic inputs): 1.79e-2.

PSUM accumulates all 21 passes at scale 4096; eviction fuses the unscale and
bias add on ScalarE: out = Identity(psum/4096 + bias).  Waves of 4+3 PSUM
tiles share each weight pass across the wave so the per-pass LDWEIGHTS
(~213ns, 256 cols, no FWL in DoubleRow) hides under 3-4 matmuls (~109ns
each) via the PE background weight buffer.
"""

import numpy as np
import ml_dtypes

import concourse.bacc as bacc
import concourse.tile as tile
import concourse.mybir as mybir
from concourse.bass_utils import run_bass_kernel_spmd

F32 = mybir.dt.float32
F8 = mybir.dt.float8e4
DR = mybir.MatmulPerfMode.DoubleRow
IDENT = mybir.ActivationFunctionType.Identity

B, CIN, COUT, H, W, K = 32, 256, 256, 56, 56, 3
NCORES = 8
BPC = B // NCORES          # images per core
WP = W + 2                 # padded row width (58)
HP = H + 3                 # padded rows (59): 1 top, 2 bottom (tail tap reads)
XF = HP * WP               # valid padded flat length per plane (3422)
XFP = 3424                 # plane pitch in SBUF/DRAM (16B aligned for DoubleRow)
OF = H * WP                # padded output flat length (3248)
NT = 7                     # output tiles per (img, cout-chunk)
NFREE = OF // NT           # 464 positions per matmul
XLOAD = 3366               # matmuls never read past 3365

SX = np.float32(16.0)      # X pre-scale (keeps e4m3 out of subnormals)
SW = np.float32(256.0)     # W pre-scale
INV_SCALE = float(1.0 / (SX * SW))

# chunks (ci, tap) with first-order fp8 compensation; 12 of 18 -> 1.79e-2
COMP = [(0, t) for t in range(9)] + [(1, t) for t in range(3)]
# 9 main DoubleRow passes (tap t, ci 0+1 in the two k-slots) + 12 comp passes
PAIRS = [("m", t) for t in range(9)] + [("c", ci, t) for (ci, t) in COMP]
NPAIR = len(PAIRS)         # 21

_CACHE = {}


def _tap_off(t):
    return (t // 3) * WP + (t % 3)


def _build():
    if "nc" in _CACHE:
        return _CACHE["nc"]
    nc = bacc.Bacc("TRN2", target_bir_lowering=False, debug=False,
                   num_swdge_queues=4)
    x_d = nc.dram_tensor("x", [BPC, 128, 4, XFP], F8, kind="ExternalInput").ap()
    w_d = nc.dram_tensor("w", [128, 2, NPAIR, 2, 128], F8,
                         kind="ExternalInput").ap()
    b_d = nc.dram_tensor("b", [COUT], F32, kind="ExternalInput").ap()
    o_d = nc.dram_tensor("o", [BPC, COUT, OF], F32, kind="ExternalOutput").ap()

    with tile.TileContext(nc) as tc:
        with (
            tc.tile_pool(name="wp", bufs=1) as wp,
            tc.tile_pool(name="xp", bufs=2) as xp,
            tc.tile_pool(name="op", bufs=2) as op,
            tc.tile_pool(name="pp", bufs=8, space="PSUM") as pp,
        ):
            bias_t = wp.tile([128, 2], F32)
            w_t = wp.tile([128, 2, NPAIR, 2, 128], F8)

            # weights + bias up front on scalar (HWDGE; idle until evictions
            # start); main-pass weights of cc=0 first so the PE can start as
            # soon as the first x slice lands
            nc.scalar.dma_start(out=w_t[:, 0:1, 0:9], in_=w_d[:, 0:1, 0:9])
            nc.scalar.dma_start(out=bias_t[:, 0:1], in_=b_d[0:128])
            nc.scalar.dma_start(out=w_t[:, 0:1, 9:NPAIR], in_=w_d[:, 0:1, 9:NPAIR])
            nc.scalar.dma_start(out=bias_t[:, 1:2], in_=b_d[128:256])
            nc.scalar.dma_start(out=w_t[:, 1:2], in_=w_d[:, 1:2])

            def load_img(img):
                # hi planes first (main passes run first), in two col slices
                x_t = xp.tile([128, 4, XFP], F8, tag="x")
                e = nc.sync
                e.dma_start(out=x_t[:, 0:2, 0:2368], in_=x_d[img, :, 0:2, 0:2368])
                e.dma_start(out=x_t[:, 2:4, 0:2368], in_=x_d[img, :, 2:4, 0:2368])
                e.dma_start(out=x_t[:, 0:2, 2368:XLOAD],
                            in_=x_d[img, :, 0:2, 2368:XLOAD])
                e.dma_start(out=x_t[:, 2:4, 2368:XLOAD],
                            in_=x_d[img, :, 2:4, 2368:XLOAD])
                return x_t

            def x_ap(x_t, p_idx, nt):
                pair = PAIRS[p_idx]
                if pair[0] == "m":
                    off = nt * NFREE + _tap_off(pair[1])
                    return x_t[:, 0:2, off:off + NFREE]          # (hi0, hi1)
                ci, t = pair[1], pair[2]
                off = nt * NFREE + _tap_off(t)
                return x_t[:, ci:ci + 3:2, off:off + NFREE]      # (hi ci, lo ci)

            WAVES = [(0, 1, 2, 3), (4, 5, 6)]
            for img in range(BPC):
                x_t = load_img(img)
                for cc in range(2):
                    o_t = op.tile([128, OF], F32, tag="o")
                    for tiles in WAVES:
                        pss = []
                        for _ in tiles:
                            ps = pp.tile([128, NFREE], F32, tag="ps")
                            pss.append(ps)
                        for p in range(NPAIR):
                            w_ap = w_t[:, cc, p]
                            for ps, nt in zip(pss, tiles):
                                nc.tensor.matmul(
                                    ps, w_ap, x_ap(x_t, p, nt),
                                    start=(p == 0), stop=(p == NPAIR - 1),
                                    perf_mode=DR,
                                )
                        for ps, nt in zip(pss, tiles):
                            # fused unscale + bias on the otherwise-idle ScalarE
                            nc.scalar.activation(
                                o_t[:, nt * NFREE:(nt + 1) * NFREE], ps, IDENT,
                                bias=bias_t[:, cc:cc + 1], scale=INV_SCALE,
                            )
                        lo = tiles[0] * NFREE
                        hi = (tiles[-1] + 1) * NFREE
                        nc.gpsimd.dma_start(
                            out=o_d[img, cc * 128:(cc + 1) * 128, lo:hi],
                            in_=o_t[:, lo:hi],
                        )
    _dedupe_ldweights(nc)
    nc.compile()
    _CACHE["nc"] = nc
    return nc


def _dedupe_ldweights(nc):
    """Shrink redundant LDWEIGHTS to one column.

    The Tile scheduler pairs every InstMatmult with a full InstLdweights
    (~135ns for DoubleRow's 256 columns), even when consecutive matmuls use
    identical weights.  LDWEIGHTS time scales with column count, so for each
    LDWEIGHTS whose source AP matches the immediately preceding one we rewrite
    the AP to a single column of the same tile: it reloads data identical to
    what the array already holds (state unchanged, bit-exact) at ~zero cost.
    """
    import concourse.mybir as mybir_
    n = 0
    for f in nc.m.functions:
        for bb in f.blocks:
            prev_key = None
            for ins in bb.instructions:
                if not isinstance(ins, mybir_.InstLdweights):
                    continue
                a = ins.ins[0]
                key = (a.memref, a.offset, tuple(tuple(d) for d in a.ap))
                if key == prev_key:
                    ap = [list(d) for d in a.ap]
                    ap[-1] = [ap[-1][0], 1]
                    a.ap = ap
                    n += 1
                else:
                    prev_key = key
    assert n, "expected redundant LDWEIGHTS to dedupe"
    return n


def make_in_maps(inp, kernel, bias):
    E4 = ml_dtypes.float8_e4m3
    x = np.asarray(inp, np.float32) * SX
    xpad = np.zeros((B, CIN, HP, WP), np.float32)
    xpad[:, :, 1:1 + H, 1:1 + W] = x
    xf = xpad.reshape(B, CIN, XF)
    xh8 = xf.astype(E4)
    xl8 = (xf - xh8.astype(np.float32)).astype(E4)
    xdev = np.zeros((B, 128, 4, XFP), E4)
    xdev[:, :, 0, :XF] = xh8[:, 0:128]
    xdev[:, :, 1, :XF] = xh8[:, 128:256]
    xdev[:, :, 2, :XF] = xl8[:, 0:128]
    xdev[:, :, 3, :XF] = xl8[:, 128:256]

    wsc = np.asarray(kernel, np.float32) * SW        # [cout, cin, kh, kw]
    wh8 = wsc.astype(E4)
    wl8 = (wsc - wh8.astype(np.float32)).astype(E4)

    def slab(src, ci, t, cc):
        kh, kw = divmod(t, 3)
        return src[cc * 128:(cc + 1) * 128, ci * 128:(ci + 1) * 128, kh, kw].T

    wdev = np.zeros((128, 2, NPAIR, 2, 128), E4)
    for cc in range(2):
        for p_idx, pair in enumerate(PAIRS):
            if pair[0] == "m":
                t = pair[1]
                wdev[:, cc, p_idx, 0] = slab(wh8, 0, t, cc)   # x slot: hi0
                wdev[:, cc, p_idx, 1] = slab(wh8, 1, t, cc)   # x slot: hi1
            else:
                ci, t = pair[1], pair[2]
                wdev[:, cc, p_idx, 0] = slab(wl8, ci, t, cc)  # x slot: hi ci
                wdev[:, cc, p_idx, 1] = slab(wh8, ci, t, cc)  # x slot: lo ci
    b_dev = np.ascontiguousarray(np.asarray(bias, np.float32))
    return [
        {"x": np.ascontiguousarray(xdev[c * BPC:(c + 1) * BPC]),
         "w": wdev, "b": b_dev}
        for c in range(NCORES)
    ]


def assemble(results):
    o = np.concatenate([results[c]["o"] for c in range(NCORES)], axis=0)
    return np.ascontiguousarray(
        o.reshape(B, COUT, H, WP)[:, :, :, :W].astype(np.float32)
    )


def kernel(inp, kernel, bias):
    nc = _build()
    in_maps = make_in_maps(inp, kernel, bias)
    r = run_bass_kernel_spmd(nc, in_maps, core_ids=list(range(NCORES)))
    return assemble(r.results)


# revision 6
# speedup vs baseline: 1.2961x; 1.2957x over previous
"""Trainium2 Bass kernel for nn_Conv2d (B=32, 256->256, 56x56, 3x3, pad=1) + bias.

Strategy
--------
Data-parallel over batch: 4 images per NeuronCore x 8 cores; weights/bias
replicated; no collectives.

Per core the conv is computed as shifted matmuls (zero-padded 58-wide rows,
output position (h,w) <-> flat h*58+w; tap (kh,kw) is a matmul against the
input shifted by kh*58+kw).  The contraction is 18 chunks (2 cin-halves x 9
taps).  Mixed precision on the PE:

- taps 0-5 (12 chunks) run in bf16: 1 cycle/row, weight loads hidden by FWL
  + the LDWEIGHTS-shrink pass below.
- taps 6-8 (6 chunks) run as pure fp8 e4m3 DoubleRow: each pass contracts
  both cin-halves at once (2 fp8 weights per PE cell, 2 MACs/cycle), so 6
  chunks cost only 3 passes.

15 passes/tile instead of 18 -> ~17% less PE stream time.  e4m3 on 6 of 18
chunks gives measured rel err 1.68e-2 (exact, deterministic inputs) vs the
2e-2 gate; the earlier all-fp8 run matched the numpy simulation to 5e-6 on
hardware, so this margin is real.

All operands are pre-scaled on the host (X~=16x, W~=256w; bf16 and e4m3 at
the same scales) so every pass accumulates into PSUM at scale 4096; eviction
fuses the unscale and bias add on ScalarE: out = Identity(psum/4096 + bias).

PSUM waves of 4+3 tiles share each weight pass across the wave; the
_dedupe_ldweights pass shrinks the per-matmul LDWEIGHTS that repeat the
previous one to a single column (a bit-exact no-op reload), leaving one full
load per weight change.
"""

import numpy as np
import ml_dtypes

import concourse.bacc as bacc
import concourse.tile as tile
import concourse.mybir as mybir
from concourse.bass_utils import run_bass_kernel_spmd

F32 = mybir.dt.float32
BF16 = mybir.dt.bfloat16
F8 = mybir.dt.float8e4
DR = mybir.MatmulPerfMode.DoubleRow
IDENT = mybir.ActivationFunctionType.Identity

B, CIN, COUT, H, W, K = 32, 256, 256, 56, 56, 3
NCORES = 8
BPC = B // NCORES          # images per core
WP = W + 2                 # padded row width (58)
HP = H + 3                 # padded rows (59): 1 top, 2 bottom (tail tap reads)
XF = HP * WP               # valid padded flat length per plane (3422)
XFP = 3424                 # plane pitch (16B-aligned slot stride for DoubleRow)
OF = H * WP                # padded output flat length (3248)
NT = 7                     # output tiles per (img, cout-chunk)
NFREE = OF // NT           # 464 positions per matmul
XLOAD = 3366               # matmuls never read past 3365

SX = np.float32(16.0)
SW = np.float32(256.0)
INV_SCALE = float(1.0 / (SX * SW))

FP8_TAPS = (6, 7, 8)       # chunks run as pure e4m3 DoubleRow (err 1.68e-2)
BF_TAPS = (0, 1, 2, 3, 4, 5)
BF_PASSES = [(t, ci) for t in BF_TAPS for ci in range(2)]   # 12
NPASS = len(BF_PASSES) + len(FP8_TAPS)                      # 15

_CACHE = {}


def _tap_off(t):
    return (t // 3) * WP + (t % 3)


def _build():
    if "nc" in _CACHE:
        return _CACHE["nc"]
    nc = bacc.Bacc("TRN2", target_bir_lowering=False, debug=False,
                   num_swdge_queues=4)
    xb_d = nc.dram_tensor("xb", [BPC, 128, 2, XFP], BF16,
                          kind="ExternalInput").ap()
    xh_d = nc.dram_tensor("xh", [BPC, 128, 2, XFP], F8,
                          kind="ExternalInput").ap()
    wb_d = nc.dram_tensor("wb", [128, 2, len(BF_PASSES), 128], BF16,
                          kind="ExternalInput").ap()
    wh_d = nc.dram_tensor("wh", [128, 2, len(FP8_TAPS), 2, 128], F8,
                          kind="ExternalInput").ap()
    b_d = nc.dram_tensor("b", [COUT], F32, kind="ExternalInput").ap()
    o_d = nc.dram_tensor("o", [BPC, COUT, OF], F32, kind="ExternalOutput").ap()

    with tile.TileContext(nc) as tc:
        with (
            tc.tile_pool(name="wp", bufs=1) as wp,
            tc.tile_pool(name="xp", bufs=2) as xp,
            tc.tile_pool(name="op", bufs=2) as op,
            tc.tile_pool(name="pp", bufs=8, space="PSUM") as pp,
        ):
            bias_t = wp.tile([128, 2], F32)
            wb_t = wp.tile([128, 2, len(BF_PASSES), 128], BF16)
            wh_t = wp.tile([128, 2, len(FP8_TAPS), 2, 128], F8)

            # weights + bias up front on scalar (idle until evictions start);
            # cc=0 first so the PE can start once the first x slice lands
            nc.scalar.dma_start(out=wh_t[:, 0:1], in_=wh_d[:, 0:1])
            nc.scalar.dma_start(out=wb_t[:, 0:1], in_=wb_d[:, 0:1])
            nc.scalar.dma_start(out=bias_t[:, 0:1], in_=b_d[0:128])
            nc.scalar.dma_start(out=wh_t[:, 1:2], in_=wh_d[:, 1:2])
            nc.scalar.dma_start(out=wb_t[:, 1:2], in_=wb_d[:, 1:2])
            nc.scalar.dma_start(out=bias_t[:, 1:2], in_=b_d[128:256])

            def load_img(img):
                xb_t = xp.tile([128, 2, XFP], BF16, tag="xb")
                xh_t = xp.tile([128, 2, XFP], F8, tag="xh")
                e = nc.sync
                e.dma_start(out=xb_t[:, :, 0:2368], in_=xb_d[img, :, :, 0:2368])
                e.dma_start(out=xh_t[:, :, 0:2368], in_=xh_d[img, :, :, 0:2368])
                e.dma_start(out=xb_t[:, :, 2368:XLOAD],
                            in_=xb_d[img, :, :, 2368:XLOAD])
                e.dma_start(out=xh_t[:, :, 2368:XLOAD],
                            in_=xh_d[img, :, :, 2368:XLOAD])
                return xb_t, xh_t

            WAVES = [(0, 1, 2, 3), (4, 5, 6)]
            for img in range(BPC):
                xb_t, xh_t = load_img(img)
                for cc in range(2):
                    o_t = op.tile([128, OF], F32, tag="o")
                    for tiles in WAVES:
                        pss = []
                        for _ in tiles:
                            ps = pp.tile([128, NFREE], F32, tag="ps")
                            pss.append(ps)
                        # fp8 DoubleRow passes: taps 6-8, both cin halves
                        for k, t in enumerate(FP8_TAPS):
                            off0 = _tap_off(t)
                            w_ap = wh_t[:, cc, k]
                            for ps, nt in zip(pss, tiles):
                                off = nt * NFREE + off0
                                nc.tensor.matmul(
                                    ps, w_ap, xh_t[:, 0:2, off:off + NFREE],
                                    start=(k == 0), stop=False, perf_mode=DR,
                                )
                        # bf16 passes: taps 0-5 x cin half
                        for p, (t, ci) in enumerate(BF_PASSES):
                            off0 = _tap_off(t)
                            w_ap = wb_t[:, cc, p]
                            last = p == len(BF_PASSES) - 1
                            for ps, nt in zip(pss, tiles):
                                off = nt * NFREE + off0
                                nc.tensor.matmul(
                                    ps, w_ap, xb_t[:, ci, off:off + NFREE],
                                    start=False, stop=last,
                                )
                        for ps, nt in zip(pss, tiles):
                            nc.scalar.activation(
                                o_t[:, nt * NFREE:(nt + 1) * NFREE], ps, IDENT,
                                bias=bias_t[:, cc:cc + 1], scale=INV_SCALE,
                            )
                        lo = tiles[0] * NFREE
                        hi = (tiles[-1] + 1) * NFREE
                        nc.gpsimd.dma_start(
                            out=o_d[img, cc * 128:(cc + 1) * 128, lo:hi],
                            in_=o_t[:, lo:hi],
                        )
    _dedupe_ldweights(nc)
    nc.compile()
    _CACHE["nc"] = nc
    return nc


def _dedupe_ldweights(nc):
    """Shrink redundant LDWEIGHTS to one column.

    The Tile scheduler pairs every InstMatmult with a full InstLdweights,
    even when consecutive matmuls use identical weights.  LDWEIGHTS time
    scales with column count, so for each LDWEIGHTS whose source AP matches
    the immediately preceding one we rewrite the AP to a single column of the
    same tile: it reloads data identical to what the array already holds
    (state unchanged, bit-exact) at ~zero cost.
    """
    n = 0
    for f in nc.m.functions:
        for bb in f.blocks:
            prev_key = None
            for ins in bb.instructions:
                if not isinstance(ins, mybir.InstLdweights):
                    continue
                a = ins.ins[0]
                key = (a.memref, a.offset, tuple(tuple(d) for d in a.ap))
                if key == prev_key:
                    ap = [list(d) for d in a.ap]
                    ap[-1] = [ap[-1][0], 1]
                    a.ap = ap
                    n += 1
                else:
                    prev_key = key
    assert n, "expected redundant LDWEIGHTS to dedupe"
    return n


def make_in_maps(inp, kernel, bias):
    E4 = ml_dtypes.float8_e4m3
    BF = ml_dtypes.bfloat16
    x = np.asarray(inp, np.float32) * SX
    xpad = np.zeros((B, CIN, HP, WP), np.float32)
    xpad[:, :, 1:1 + H, 1:1 + W] = x
    xf = xpad.reshape(B, CIN, XF)
    xb = np.zeros((B, 128, 2, XFP), BF)
    xh = np.zeros((B, 128, 2, XFP), E4)
    for ci in range(2):
        xb[:, :, ci, :XF] = xf[:, ci * 128:(ci + 1) * 128]
        xh[:, :, ci, :XF] = xf[:, ci * 128:(ci + 1) * 128]

    wsc = np.asarray(kernel, np.float32) * SW        # [cout, cin, kh, kw]

    def slab(src, ci, t, cc):
        kh, kw = divmod(t, 3)
        return src[cc * 128:(cc + 1) * 128, ci * 128:(ci + 1) * 128, kh, kw].T

    wb = np.zeros((128, 2, len(BF_PASSES), 128), BF)
    wh = np.zeros((128, 2, len(FP8_TAPS), 2, 128), E4)
    wsc_bf = wsc.astype(BF).astype(np.float32)
    wsc_e4 = wsc.astype(E4).astype(np.float32)
    for cc in range(2):
        for p, (t, ci) in enumerate(BF_PASSES):
            wb[:, cc, p] = slab(wsc_bf, ci, t, cc)
        for k, t in enumerate(FP8_TAPS):
            wh[:, cc, k, 0] = slab(wsc_e4, 0, t, cc)
            wh[:, cc, k, 1] = slab(wsc_e4, 1, t, cc)
    b_dev = np.ascontiguousarray(np.asarray(bias, np.float32))
    return [
        {"xb": np.ascontiguousarray(xb[c * BPC:(c + 1) * BPC]),
         "xh": np.ascontiguousarray(xh[c * BPC:(c + 1) * BPC]),
         "wb": wb, "wh": wh, "b": b_dev}
        for c in range(NCORES)
    ]


def assemble(results):
    o = np.concatenate([results[c]["o"] for c in range(NCORES)], axis=0)
    return np.ascontiguousarray(
        o.reshape(B, COUT, H, WP)[:, :, :, :W].astype(np.float32)
    )


def kernel(inp, kernel, bias):
    nc = _build()
    in_maps = make_in_maps(inp, kernel, bias)
    r = run_bass_kernel_spmd(nc, in_maps, core_ids=list(range(NCORES)))
    return assemble(r.results)


# revision 7
# speedup vs baseline: 1.3145x; 1.0141x over previous
"""Trainium2 Bass kernel for nn_Conv2d (B=32, 256->256, 56x56, 3x3, pad=1) + bias.

Strategy
--------
Data-parallel over batch: 4 images per NeuronCore x 8 cores; weights/bias
replicated; no collectives.

Per core the conv is computed as shifted matmuls (zero-padded 58-wide rows,
output position (h,w) <-> flat h*58+w; tap (kh,kw) is a matmul against the
input shifted by kh*58+kw).  The contraction is 18 chunks (2 cin-halves x 9
taps).  Mixed precision on the PE:

- taps 0-5 (12 chunks) run in bf16: 1 cycle/row, weight loads hidden by FWL
  + the LDWEIGHTS-shrink pass below.
- taps 6-8 (6 chunks) run as pure fp8 e4m3 DoubleRow: each pass contracts
  both cin-halves at once (2 fp8 weights per PE cell, 2 MACs/cycle), so 6
  chunks cost only 3 passes.

15 passes/tile instead of 18 -> ~17% less PE stream time.  e4m3 on 6 of 18
chunks gives measured rel err 1.68e-2 (exact, deterministic inputs) vs the
2e-2 gate; the earlier all-fp8 run matched the numpy simulation to 5e-6 on
hardware, so this margin is real.

All operands are pre-scaled on the host (X~=16x, W~=256w; bf16 and e4m3 at
the same scales) so every pass accumulates into PSUM at scale 4096; eviction
fuses the unscale and bias add on ScalarE: out = Identity(psum/4096 + bias).

PSUM waves of 4+3 tiles share each weight pass across the wave; the
_dedupe_ldweights pass shrinks the per-matmul LDWEIGHTS that repeat the
previous one to a single column (a bit-exact no-op reload), leaving one full
load per weight change.
"""

import numpy as np
import ml_dtypes

import concourse.bacc as bacc
import concourse.tile as tile
import concourse.mybir as mybir
from concourse.bass_utils import run_bass_kernel_spmd

F32 = mybir.dt.float32
BF16 = mybir.dt.bfloat16
F8 = mybir.dt.float8e4
F16 = mybir.dt.float16
DR = mybir.MatmulPerfMode.DoubleRow
IDENT = mybir.ActivationFunctionType.Identity

B, CIN, COUT, H, W, K = 32, 256, 256, 56, 56, 3
NCORES = 8
BPC = B // NCORES          # images per core
WP = W + 2                 # padded row width (58)
HP = H + 3                 # padded rows (59): 1 top, 2 bottom (tail tap reads)
XF = HP * WP               # valid padded flat length per plane (3422)
XFP = 3424                 # plane pitch (16B-aligned slot stride for DoubleRow)
OF = H * WP                # padded output flat length (3248)
NT = 7                     # output tiles per (img, cout-chunk)
NFREE = OF // NT           # 464 positions per matmul
XLOAD = 3366               # matmuls never read past 3365

SX = np.float32(16.0)
SW = np.float32(256.0)
INV_SCALE = float(1.0 / (SX * SW))

FP8_TAPS = (6, 7, 8)       # chunks run as pure e4m3 DoubleRow (err 1.68e-2)
BF_TAPS = (0, 1, 2, 3, 4, 5)
BF_PASSES = [(t, ci) for t in BF_TAPS for ci in range(2)]   # 12
NPASS = len(BF_PASSES) + len(FP8_TAPS)                      # 15

_CACHE = {}


def _tap_off(t):
    return (t // 3) * WP + (t % 3)


def _build():
    if "nc" in _CACHE:
        return _CACHE["nc"]
    nc = bacc.Bacc("TRN2", target_bir_lowering=False, debug=False,
                   num_swdge_queues=4)
    xb_d = nc.dram_tensor("xb", [BPC, 128, 2, XFP], BF16,
                          kind="ExternalInput").ap()
    xh_d = nc.dram_tensor("xh", [BPC, 128, 2, XFP], F8,
                          kind="ExternalInput").ap()
    wb_d = nc.dram_tensor("wb", [128, 2, len(BF_PASSES), 128], BF16,
                          kind="ExternalInput").ap()
    wh_d = nc.dram_tensor("wh", [128, 2, len(FP8_TAPS), 2, 128], F8,
                          kind="ExternalInput").ap()
    b_d = nc.dram_tensor("b", [COUT], F32, kind="ExternalInput").ap()
    o_d = nc.dram_tensor("o", [BPC, COUT, OF], F16, kind="ExternalOutput").ap()

    with tile.TileContext(nc) as tc:
        with (
            tc.tile_pool(name="wp", bufs=1) as wp,
            tc.tile_pool(name="xp", bufs=2) as xp,
            tc.tile_pool(name="op", bufs=2) as op,
            tc.tile_pool(name="pp", bufs=8, space="PSUM") as pp,
        ):
            bias_t = wp.tile([128, 2], F32)
            wb_t = wp.tile([128, 2, len(BF_PASSES), 128], BF16)
            wh_t = wp.tile([128, 2, len(FP8_TAPS), 2, 128], F8)

            # weights + bias up front on scalar (idle until evictions start);
            # cc=0 first so the PE can start once the first x slice lands
            nc.scalar.dma_start(out=wh_t[:, 0:1], in_=wh_d[:, 0:1])
            nc.scalar.dma_start(out=wb_t[:, 0:1], in_=wb_d[:, 0:1])
            nc.scalar.dma_start(out=bias_t[:, 0:1], in_=b_d[0:128])
            nc.scalar.dma_start(out=wh_t[:, 1:2], in_=wh_d[:, 1:2])
            nc.scalar.dma_start(out=wb_t[:, 1:2], in_=wb_d[:, 1:2])
            nc.scalar.dma_start(out=bias_t[:, 1:2], in_=b_d[128:256])

            def load_img(img):
                xb_t = xp.tile([128, 2, XFP], BF16, tag="xb")
                xh_t = xp.tile([128, 2, XFP], F8, tag="xh")
                e = nc.sync
                e.dma_start(out=xb_t[:, :, 0:2368], in_=xb_d[img, :, :, 0:2368])
                e.dma_start(out=xh_t[:, :, 0:2368], in_=xh_d[img, :, :, 0:2368])
                e.dma_start(out=xb_t[:, :, 2368:XLOAD],
                            in_=xb_d[img, :, :, 2368:XLOAD])
                e.dma_start(out=xh_t[:, :, 2368:XLOAD],
                            in_=xh_d[img, :, :, 2368:XLOAD])
                return xb_t, xh_t

            WAVES = [(0, 1, 2, 3), (4, 5, 6)]
            for img in range(BPC):
                xb_t, xh_t = load_img(img)
                for cc in range(2):
                    o_t = op.tile([128, OF], F16, tag="o")
                    for tiles in WAVES:
                        pss = []
                        for _ in tiles:
                            ps = pp.tile([128, NFREE], F32, tag="ps")
                            pss.append(ps)
                        # fp8 DoubleRow passes: taps 6-8, both cin halves
                        for k, t in enumerate(FP8_TAPS):
                            off0 = _tap_off(t)
                            w_ap = wh_t[:, cc, k]
                            for ps, nt in zip(pss, tiles):
                                off = nt * NFREE + off0
                                nc.tensor.matmul(
                                    ps, w_ap, xh_t[:, 0:2, off:off + NFREE],
                                    start=(k == 0), stop=False, perf_mode=DR,
                                )
                        # bf16 passes: taps 0-5 x cin half
                        for p, (t, ci) in enumerate(BF_PASSES):
                            off0 = _tap_off(t)
                            w_ap = wb_t[:, cc, p]
                            last = p == len(BF_PASSES) - 1
                            for ps, nt in zip(pss, tiles):
                                off = nt * NFREE + off0
                                nc.tensor.matmul(
                                    ps, w_ap, xb_t[:, ci, off:off + NFREE],
                                    start=False, stop=last,
                                )
                        for ps, nt in zip(pss, tiles):
                            nc.scalar.activation(
                                o_t[:, nt * NFREE:(nt + 1) * NFREE], ps, IDENT,
                                bias=bias_t[:, cc:cc + 1], scale=INV_SCALE,
                            )
                        lo = tiles[0] * NFREE
                        hi = (tiles[-1] + 1) * NFREE
                        nc.gpsimd.dma_start(
                            out=o_d[img, cc * 128:(cc + 1) * 128, lo:hi],
                            in_=o_t[:, lo:hi],
                        )
    _dedupe_ldweights(nc)
    nc.compile()
    _CACHE["nc"] = nc
    return nc


def _dedupe_ldweights(nc):
    """Shrink redundant LDWEIGHTS to one column.

    The Tile scheduler pairs every InstMatmult with a full InstLdweights,
    even when consecutive matmuls use identical weights.  LDWEIGHTS time
    scales with column count, so for each LDWEIGHTS whose source AP matches
    the immediately preceding one we rewrite the AP to a single column of the
    same tile: it reloads data identical to what the array already holds
    (state unchanged, bit-exact) at ~zero cost.
    """
    n = 0
    for f in nc.m.functions:
        for bb in f.blocks:
            prev_key = None
            for ins in bb.instructions:
                if not isinstance(ins, mybir.InstLdweights):
                    continue
                a = ins.ins[0]
                key = (a.memref, a.offset, tuple(tuple(d) for d in a.ap))
                if key == prev_key:
                    ap = [list(d) for d in a.ap]
                    ap[-1] = [ap[-1][0], 1]
                    a.ap = ap
                    n += 1
                else:
                    prev_key = key
    assert n, "expected redundant LDWEIGHTS to dedupe"
    return n


def make_in_maps(inp, kernel, bias):
    E4 = ml_dtypes.float8_e4m3
    BF = ml_dtypes.bfloat16
    x = np.asarray(inp, np.float32) * SX
    xpad = np.zeros((B, CIN, HP, WP), np.float32)
    xpad[:, :, 1:1 + H, 1:1 + W] = x
    xf = xpad.reshape(B, CIN, XF)
    xb = np.zeros((B, 128, 2, XFP), BF)
    xh = np.zeros((B, 128, 2, XFP), E4)
    for ci in range(2):
        xb[:, :, ci, :XF] = xf[:, ci * 128:(ci + 1) * 128]
        xh[:, :, ci, :XF] = xf[:, ci * 128:(ci + 1) * 128]

    wsc = np.asarray(kernel, np.float32) * SW        # [cout, cin, kh, kw]

    def slab(src, ci, t, cc):
        kh, kw = divmod(t, 3)
        return src[cc * 128:(cc + 1) * 128, ci * 128:(ci + 1) * 128, kh, kw].T

    wb = np.zeros((128, 2, len(BF_PASSES), 128), BF)
    wh = np.zeros((128, 2, len(FP8_TAPS), 2, 128), E4)
    wsc_bf = wsc.astype(BF).astype(np.float32)
    wsc_e4 = wsc.astype(E4).astype(np.float32)
    for cc in range(2):
        for p, (t, ci) in enumerate(BF_PASSES):
            wb[:, cc, p] = slab(wsc_bf, ci, t, cc)
        for k, t in enumerate(FP8_TAPS):
            wh[:, cc, k, 0] = slab(wsc_e4, 0, t, cc)
            wh[:, cc, k, 1] = slab(wsc_e4, 1, t, cc)
    b_dev = np.ascontiguousarray(np.asarray(bias, np.float32))
    return [
        {"xb": np.ascontiguousarray(xb[c * BPC:(c + 1) * BPC]),
         "xh": np.ascontiguousarray(xh[c * BPC:(c + 1) * BPC]),
         "wb": wb, "wh": wh, "b": b_dev}
        for c in range(NCORES)
    ]


def assemble(results):
    o = np.concatenate([results[c]["o"] for c in range(NCORES)], axis=0)
    return np.ascontiguousarray(
        o.reshape(B, COUT, H, WP)[:, :, :, :W].astype(np.float32)
    )


def kernel(inp, kernel, bias):
    nc = _build()
    in_maps = make_in_maps(inp, kernel, bias)
    r = run_bass_kernel_spmd(nc, in_maps, core_ids=list(range(NCORES)))
    return assemble(r.results)
